# revision 5
# baseline (speedup 1.0000x reference)
"""Trainium2 Bass kernel for CompactedViTAttention.

Full (unsharded) inputs in, full output out. Internally: data-parallel over
batch across 8 NeuronCores (2 batches per core), LN gamma/beta folded into the
QKV projection weights host-side, bf16 matmul operands with fp32 PSUM
accumulation, S^T-layout attention with a ones-column denominator trick.
"""

import sys

sys.path.insert(0, "/opt/trn_rl_repo")

import numpy as np
import ml_dtypes

import concourse.bass as bass
import concourse.tile as tile
import concourse.mybir as mybir
from concourse import bacc
from concourse.bass_utils import run_bass_kernel_spmd

F32 = mybir.dt.float32
BF16 = mybir.dt.bfloat16

N_CORES = 8
B, N, H = 16, 1025, 768
NH, DH = 6, 64
AH = NH * DH          # 384
BPC = B // N_CORES    # batches per core = 2
T = BPC * N           # tokens per core = 2050
EPS = 1e-12
K6 = H // 128         # 6 hidden chunks
M3 = AH // 128        # 3 all-head chunks
VW = NH * (DH + 1)    # 390: V with interleaved ones columns

TOK_TILES = [(i * 128, min(128, T - i * 128)) for i in range((T + 127) // 128)]
QC5 = [(o, min(512, T - o)) for o in range(0, T, 512)]
KT9 = [(o, min(128, N - o)) for o in range(0, N, 128)]
QCB = [(0, 512), (512, 512)]  # per-batch q chunks (col 1024 handled separately)
TPAD = 2176  # normT free-dim stride: 2176*2B is 32B-aligned (xbar transpose dest requirement)


def _build():
    nc = bacc.Bacc("TRN2", target_bir_lowering=False, debug=False)

    x_d = nc.dram_tensor("x", [T, H], F32, kind="ExternalInput")
    wq_d = nc.dram_tensor("wq", [H, AH], BF16, kind="ExternalInput")
    wk_d = nc.dram_tensor("wk", [H, AH], BF16, kind="ExternalInput")
    wv_d = nc.dram_tensor("wv", [H, VW], BF16, kind="ExternalInput")
    bqk_d = nc.dram_tensor("bqk", [128, 6], F32, kind="ExternalInput")
    bv_d = nc.dram_tensor("bv", [1, VW], BF16, kind="ExternalInput")
    wo_d = nc.dram_tensor("wo", [AH, H], BF16, kind="ExternalInput")
    bo_d = nc.dram_tensor("bo", [1, H], BF16, kind="ExternalInput")
    out_d = nc.dram_tensor("out", [T, H], F32, kind="ExternalOutput")

    with tile.TileContext(nc) as tc:
        with (
            tc.tile_pool(name="consts", bufs=1) as consts,
            tc.tile_pool(name="big", bufs=1) as big,
            tc.tile_pool(name="dram", bufs=1, space="DRAM") as dramp,
            tc.tile_pool(name="psA", bufs=2, space="PSUM") as pst,
            tc.tile_pool(name="psB", bufs=4, space="PSUM") as psmm,
        ):
            # ---- constants ----
            wq_sb = consts.tile([128, K6, AH], BF16, tag="wq")
            wk_sb = consts.tile([128, K6, AH], BF16, tag="wk")
            wv_sb = consts.tile([128, K6, VW], BF16, tag="wv")
            wo_sb = consts.tile([128, M3, H], BF16, tag="wo")
            bqk_sb = consts.tile([128, 6], F32, tag="bqk")
            bv_sb = consts.tile([1, VW], BF16, tag="bv")
            bo_sb = consts.tile([1, H], BF16, tag="bo")
            ones_sb = consts.tile([1, 128], BF16, tag="ones")
            eps_sb = consts.tile([128, 1], F32, tag="eps")

            nc.sync.dma_start(out=wq_sb, in_=wq_d.ap().rearrange("(k p) n -> p k n", p=128))
            nc.sync.dma_start(out=wk_sb, in_=wk_d.ap().rearrange("(k p) n -> p k n", p=128))
            nc.sync.dma_start(out=wv_sb, in_=wv_d.ap().rearrange("(k p) n -> p k n", p=128))
            nc.sync.dma_start(out=wo_sb, in_=wo_d.ap().rearrange("(k p) n -> p k n", p=128))
            nc.sync.dma_start(out=bqk_sb, in_=bqk_d.ap())
            nc.sync.dma_start(out=bv_sb, in_=bv_d.ap())
            nc.sync.dma_start(out=bo_sb, in_=bo_d.ap())
            nc.vector.memset(ones_sb, 1.0)
            nc.vector.memset(eps_sb, EPS)

            # ---- big persistent tensors ----
            qt = big.tile([128, M3, T], BF16, tag="qt")
            kt = big.tile([128, M3, T], BF16, tag="kt")
            ctxT = big.tile([128, M3, T], BF16, tag="ctxT")
            v_tiles = [
                [big.tile([128, VW], BF16, tag=f"v{b}_{ik}", name=f"v{b}_{ik}") for ik in range(len(KT9))]
                for b in range(BPC)
            ]
            scratch = dramp.tile([T, H], BF16, tag="scratch")

            phaseA = tc.alloc_tile_pool(name="phaseA", bufs=1)
            ln = tc.alloc_tile_pool(name="ln", bufs=3)
            statp = tc.alloc_tile_pool(name="stat", bufs=4)
            normT = phaseA.tile([128, K6, TPAD], BF16, tag="normT")

            # ---- phase 1: layernorm (token-major) -> scratch (bf16) ----
            for i, (to, p) in enumerate(TOK_TILES):
                x_sb = ln.tile([128, H], F32, tag="x")
                nc.gpsimd.dma_start(out=x_sb[:p], in_=x_d.ap()[to : to + p, :])
                stats = statp.tile([128, 3, 6], F32, tag="stats")
                xg = x_sb.rearrange("p (s c) -> p s c", s=3)
                for s in range(3):
                    nc.vector.bn_stats(out=stats[:p, s, :], in_=xg[:p, s, :])
                mv = statp.tile([128, 2], F32, tag="mv")
                nc.vector.bn_aggr(out=mv[:p], in_=stats[:p])
                sd = statp.tile([128, 1], F32, tag="sd")
                nc.scalar.activation(
                    out=sd[:p], in_=mv[:p, 1:2],
                    func=mybir.ActivationFunctionType.Sqrt, bias=eps_sb[:p],
                )
                rstd = statp.tile([128, 1], F32, tag="rstd")
                nc.vector.reciprocal(out=rstd[:p], in_=sd[:p])
                nb = ln.tile([128, H], BF16, tag="nb")
                nc.vector.tensor_scalar(
                    out=nb[:p], in0=x_sb[:p],
                    scalar1=mv[:p, 0:1], scalar2=rstd[:p],
                    op0=mybir.AluOpType.subtract, op1=mybir.AluOpType.mult,
                )
                nc.gpsimd.dma_start(out=scratch[to : to + p, :], in_=nb[:p])

            # ---- phase 2: transpose scratch -> normT [hid, tok] ----
            for k in range(K6):
                for piece in range(4):
                    r0 = piece * 512
                    nc.sync.dma_start(
                        out=normT[:, k, r0 : r0 + 512],
                        in_=scratch[r0 : r0 + 512, k * 128 : (k + 1) * 128],
                        transpose=True,
                    )
                # last 2 rows (2048:2050) via small strided copy
                nc.sync.dma_start(
                    out=normT[:, k, 2048:2050],
                    in_=scratch[2048:2050, k * 128 : (k + 1) * 128].rearrange("a b -> b a"),
                )

            # ---- phase 3: QKV projections ----
            for qco, qcs in QC5:
                for m in range(M3):
                    psq = psmm.tile([128, 512], F32, tag="mm")
                    for k in range(K6):
                        nc.tensor.matmul(
                            out=psq[:, :qcs],
                            lhsT=wq_sb[:, k, m * 128 : (m + 1) * 128],
                            rhs=normT[:, k, qco : qco + qcs],
                            start=(k == 0), stop=(k == K6 - 1),
                        )
                    nc.scalar.activation(
                        out=qt[:, m, qco : qco + qcs], in_=psq[:, :qcs],
                        func=mybir.ActivationFunctionType.Identity,
                        bias=bqk_sb[:, m : m + 1],
                    )
                    psk = psmm.tile([128, 512], F32, tag="mm")
                    for k in range(K6):
                        nc.tensor.matmul(
                            out=psk[:, :qcs],
                            lhsT=wk_sb[:, k, m * 128 : (m + 1) * 128],
                            rhs=normT[:, k, qco : qco + qcs],
                            start=(k == 0), stop=(k == K6 - 1),
                        )
                    nc.scalar.activation(
                        out=kt[:, m, qco : qco + qcs], in_=psk[:, :qcs],
                        func=mybir.ActivationFunctionType.Identity,
                        bias=bqk_sb[:, 3 + m : 4 + m],
                    )
            for b in range(BPC):
                bN = b * N
                for ik, (kto, kts) in enumerate(KT9):
                    psv = psmm.tile([128, 512], F32, tag="mm")
                    for k in range(K6):
                        nc.tensor.matmul(
                            out=psv[:kts, :VW],
                            lhsT=normT[:, k, bN + kto : bN + kto + kts],
                            rhs=wv_sb[:, k, :],
                            start=(k == 0), stop=False,
                        )
                    nc.tensor.matmul(
                        out=psv[:kts, :VW],
                        lhsT=ones_sb[:, :kts], rhs=bv_sb,
                        start=False, stop=True,
                    )
                    nc.vector.tensor_copy(out=v_tiles[b][ik][:kts, :], in_=psv[:kts, :VW])

            statp.release()
            ln.release()
            phaseA.release()
            attnp = tc.alloc_tile_pool(name="attn", bufs=36)
            rbp = tc.alloc_tile_pool(name="rb", bufs=2)
            outpool = tc.alloc_tile_pool(name="outp", bufs=3)

            # ---- phase 4: attention (pairs of heads; S^T layout) ----
            pairs = [(b, j) for b in range(BPC) for j in range(M3)]

            def emit_st(b, j):
                """S^T + exp for head pair (2j, 2j+1) of batch b. Returns expS tiles."""
                bN = b * N
                exps = [[], []]
                for kto, kts in KT9:
                    stp = [pst.tile([128, 1024], F32, tag="st", name=f"st{_h}") for _h in range(2)]
                    c1p = [psmm.tile([128, 512], F32, tag="mm", name=f"c1{_h}") for _h in range(2)]
                    for hh in range(2):
                        pr = (hh * 64, (hh + 1) * 64)
                        tp = (hh * 64, 0)
                        for qco, qcs in QCB:
                            nc.tensor.matmul(
                                out=stp[hh][:kts, qco : qco + qcs],
                                lhsT=kt[pr[0] : pr[1], j, bN + kto : bN + kto + kts],
                                rhs=qt[pr[0] : pr[1], j, bN + qco : bN + qco + qcs],
                                start=True, stop=True, tile_position=tp,
                            )
                        nc.tensor.matmul(
                            out=c1p[hh][:kts, 0:1],
                            lhsT=kt[pr[0] : pr[1], j, bN + kto : bN + kto + kts],
                            rhs=qt[pr[0] : pr[1], j, bN + 1024 : bN + 1025],
                            start=True, stop=True, tile_position=tp,
                        )
                        e = attnp.tile([128, N], BF16, tag="expS")
                        nc.scalar.activation(
                            out=e[:kts, 0:1024], in_=stp[hh][:kts, :],
                            func=mybir.ActivationFunctionType.Exp, scale=0.125,
                        )
                        nc.scalar.activation(
                            out=e[:kts, 1024:1025], in_=c1p[hh][:kts, 0:1],
                            func=mybir.ActivationFunctionType.Exp, scale=0.125,
                        )
                        exps[hh].append(e)
                return exps

            def emit_pv(b, j, exps):
                bN = b * N
                for hh in range(2):
                    h = 2 * j + hh
                    pr = (hh * 64, (hh + 1) * 64)
                    recip = rbp.tile([1, N], F32, tag="recip")
                    for qco, qcs in QCB + [(1024, 1)]:
                        pv = psmm.tile([128, 512], F32, tag="mm")
                        for ik, (kto, kts) in enumerate(KT9):
                            nc.tensor.matmul(
                                out=pv[0:65, :qcs],
                                lhsT=v_tiles[b][ik][:kts, h * 65 : (h + 1) * 65],
                                rhs=exps[hh][ik][:kts, qco : qco + qcs],
                                start=(ik == 0), stop=(ik == len(KT9) - 1),
                            )
                        nc.vector.reciprocal(
                            out=recip[0:1, qco : qco + qcs], in_=pv[64:65, :qcs]
                        )
                        nc.vector.tensor_copy(
                            out=ctxT[pr[0] : pr[1], j, bN + qco : bN + qco + qcs],
                            in_=pv[0:64, :qcs],
                        )
                    rbc = rbp.tile([128, N], F32, tag="rbc")
                    nc.gpsimd.partition_broadcast(rbc, recip)
                    nc.vector.tensor_mul(
                        out=ctxT[pr[0] : pr[1], j, bN : bN + N],
                        in0=ctxT[pr[0] : pr[1], j, bN : bN + N],
                        in1=rbc[pr[0] : pr[1], :],
                    )

            prev = None
            for pb, pj in pairs:
                exps = emit_st(pb, pj)
                if prev is not None:
                    emit_pv(*prev)
                prev = (pb, pj, exps)
            emit_pv(*prev)

            # ---- phase 5: output projection + residual ----
            for to, p in TOK_TILES:
                po1 = psmm.tile([128, 512], F32, tag="mm")
                po2 = psmm.tile([128, 512], F32, tag="mm")
                for k in range(M3):
                    nc.tensor.matmul(
                        out=po1[:p, :512],
                        lhsT=ctxT[:, k, to : to + p], rhs=wo_sb[:, k, 0:512],
                        start=(k == 0), stop=False,
                    )
                    nc.tensor.matmul(
                        out=po2[:p, :256],
                        lhsT=ctxT[:, k, to : to + p], rhs=wo_sb[:, k, 512:768],
                        start=(k == 0), stop=False,
                    )
                nc.tensor.matmul(
                    out=po1[:p, :512], lhsT=ones_sb[:, :p], rhs=bo_sb[:, 0:512],
                    start=False, stop=True,
                )
                nc.tensor.matmul(
                    out=po2[:p, :256], lhsT=ones_sb[:, :p], rhs=bo_sb[:, 512:768],
                    start=False, stop=True,
                )
                x2 = outpool.tile([128, H], F32, tag="x2")
                nc.gpsimd.dma_start(out=x2[:p], in_=x_d.ap()[to : to + p, :])
                osb = outpool.tile([128, H], F32, tag="osb")
                nc.vector.tensor_add(out=osb[:p, 0:512], in0=po1[:p, :512], in1=x2[:p, 0:512])
                nc.vector.tensor_add(out=osb[:p, 512:768], in0=po2[:p, :256], in1=x2[:p, 512:768])
                nc.sync.dma_start(out=out_d.ap()[to : to + p, :], in_=osb[:p])

            outpool.release()
            rbp.release()
            attnp.release()

    nc.compile()
    return nc


_NC_CACHE = {}


def _get_nc():
    if "nc" not in _NC_CACHE:
        _NC_CACHE["nc"] = _build()
    return _NC_CACHE["nc"]


def make_in_maps(hidden_states, wq, bq, wk, bk, wv, bv, wo, bo, ln_g, ln_b):
    bf = ml_dtypes.bfloat16
    g = ln_g.astype(np.float64)
    bb = ln_b.astype(np.float64)
    wq64, wk64, wv64 = (w.astype(np.float64) for w in (wq, wk, wv))

    wq_g = (wq64 * g[:, None]).astype(np.float32)
    wk_g = (wk64 * g[:, None]).astype(np.float32)
    wv_g = (wv64 * g[:, None]).astype(np.float32)
    bq_eff = (bq.astype(np.float64) + bb @ wq64).astype(np.float32)
    bk_eff = (bk.astype(np.float64) + bb @ wk64).astype(np.float32)
    bv_eff = (bv.astype(np.float64) + bb @ wv64).astype(np.float32)

    wv_pad = np.zeros((H, VW), np.float32)
    bv_pad = np.zeros((1, VW), np.float32)
    for h in range(NH):
        wv_pad[:, h * 65 : h * 65 + 64] = wv_g[:, h * 64 : (h + 1) * 64]
        bv_pad[0, h * 65 : h * 65 + 64] = bv_eff[h * 64 : (h + 1) * 64]
        bv_pad[0, h * 65 + 64] = 1.0

    bqk = np.stack(
        [bq_eff[m * 128 : (m + 1) * 128] for m in range(M3)]
        + [bk_eff[m * 128 : (m + 1) * 128] for m in range(M3)],
        axis=1,
    ).astype(np.float32)

    common = {
        "wq": wq_g.astype(bf),
        "wk": wk_g.astype(bf),
        "wv": wv_pad.astype(bf),
        "bqk": bqk,
        "bv": bv_pad.astype(bf),
        "wo": wo.astype(bf),
        "bo": bo.reshape(1, H).astype(bf),
    }
    in_maps = []
    for c in range(N_CORES):
        shard = np.ascontiguousarray(
            hidden_states[c * BPC : (c + 1) * BPC].reshape(T, H).astype(np.float32)
        )
        in_maps.append({"x": shard, **common})
    return in_maps


def kernel(hidden_states, wq, bq, wk, bk, wv, bv, wo, bo, ln_g, ln_b):
    nc = _get_nc()
    in_maps = make_in_maps(hidden_states, wq, bq, wk, bk, wv, bv, wo, bo, ln_g, ln_b)
    res = run_bass_kernel_spmd(nc, in_maps, core_ids=list(range(N_CORES)))
    out = np.concatenate(
        [res.results[c]["out"].reshape(BPC, N, H) for c in range(N_CORES)], axis=0
    )
    return out.astype(hidden_states.dtype)


if __name__ == "__main__":
    rng = np.random.default_rng(0)
    hs = rng.standard_normal((B, N, H), dtype=np.float32)
    s_in = 1.0 / np.sqrt(H)
    s_ah = 1.0 / np.sqrt(AH)
    inputs = dict(
        hidden_states=hs,
        wq=rng.standard_normal((H, AH), dtype=np.float32) * s_in,
        bq=np.zeros(AH, np.float32),
        wk=rng.standard_normal((H, AH), dtype=np.float32) * s_in,
        bk=np.zeros(AH, np.float32),
        wv=rng.standard_normal((H, AH), dtype=np.float32) * s_in,
        bv=np.zeros(AH, np.float32),
        wo=rng.standard_normal((AH, H), dtype=np.float32) * s_ah,
        bo=np.zeros(H, np.float32),
        ln_g=np.ones(H, np.float32),
        ln_b=np.zeros(H, np.float32),
    )
    out = kernel(**inputs)
    print("out", out.shape, out.dtype, np.abs(out).max())
